# revision 1
# baseline (speedup 1.0000x reference)
"""Causal self-attention on 8 TRN2 NeuronCores.

Sharding: data-parallel over batch (2) x tensor-parallel over heads (4 heads
per core). Core c handles batch c//4, heads 4*(c%4)..4*(c%4)+3 — i.e. columns
[256*g, 256*(g+1)) of wq/wk/wv and rows [256*g, 256*(g+1)) of wo. Each core
returns a partial output [2048, 1024]; the host sums the 4 partials of each
batch and adds the (bv @ wo + bo) correction (exact because softmax rows sum
to 1).

Per-core kernel (Tile framework, fully unrolled, fp32 storage / fp32r matmul,
software-pipelined emission so PE filler work hides under the exp-bound
attention phase):
  1. x [2048,1024] -> PE-transpose -> xT chunks. qT/kT [256,2048] projected
     with xT as the moving operand (j on partitions; q scaled by 1/8 + bq,
     k + bk fused into the psum->sbuf move). v projected in natural [t, j]
     layout (xT chunk as the stationary) straight into v_aug, which carries
     a ones column per head ([128, 65] groups) so the AV matmul also produces
     the softmax denominator in row 64.
  2. Attention per (head-pair, 512-wide i-block), scores kept TRANSPOSED
     ([l-chunk=128, i=512]) so the softmax reduction lands on the matmul and
     the AV/out-proj matmuls need no further transposes. The two heads of a
     pair occupy disjoint PE row groups (K=64 at rows 0-63/64-127) and their
     score matmuls run concurrently; one [128,1024] exp covers both. Causal:
     chunks above the diagonal are skipped, diagonal chunks compute only the
     live column range and get exp() zeroed via gpsimd.affine_select.
     Normalization: DVE reciprocal of psum row 64, gpsimd partition_broadcast,
     DVE multiply.
  3. y = attn_outT.T @ wo accumulated over the 2 local j-chunks, per 128-token
     tile, DMA'd out.
  4. Schedule: attention for block i is ACT(exp)-bound, so the next block's
     transposes/projections and the previous block's out-projection are
     emitted as interleaved filler units; PSUM = 2x[128,1024] score pairs +
     2x[128,512] AV + 2x[128,512] fillers = 8 banks.
"""

import sys

import numpy as np

if "/opt/trn_rl_repo" not in sys.path:
    sys.path.insert(0, "/opt/trn_rl_repo")

import concourse.mybir as mybir
import concourse.tile as tile
from concourse import bacc
from concourse.bass_utils import run_bass_kernel_spmd

# Problem shapes (hardcoded per contract)
B, S, D = 2, 2048, 1024
H, DH = 16, 64
NCORES = 8
GROUPS = 4                  # tensor-parallel groups per batch
HL = H // GROUPS            # 4 local heads
JC = HL * DH                # 256 local head columns
T = S                       # tokens per core (one batch element)

P = 128                     # partitions
TS = 512                    # token block (projection granularity)
NTB = T // TS               # 4 token blocks
NDC = D // P                # 8 contraction chunks
IB = 512                    # attention i-block (query positions)
LCH = P                     # attention l-chunk (key positions)
VA = DH + 1                 # v_aug columns per head (ones column appended)

FP = mybir.dt.float32
FPR = mybir.dt.float32r

_CACHE = {}


def build_nc():
    nc = bacc.Bacc("TRN2", target_bir_lowering=False, debug=False)

    x = nc.dram_tensor("x", [T, D], FPR, kind="ExternalInput")
    wq = nc.dram_tensor("wq", [D, JC], FPR, kind="ExternalInput")
    wk = nc.dram_tensor("wk", [D, JC], FPR, kind="ExternalInput")
    wv = nc.dram_tensor("wv", [D, JC], FPR, kind="ExternalInput")
    wo = nc.dram_tensor("wo", [JC, D], FPR, kind="ExternalInput")
    bq = nc.dram_tensor("bq", [JC, 1], FP, kind="ExternalInput")
    bk = nc.dram_tensor("bk", [JC, 1], FP, kind="ExternalInput")
    y = nc.dram_tensor("y", [T, D], FP, kind="ExternalOutput")

    with tile.TileContext(nc) as tc:
        import contextlib

        with contextlib.ExitStack() as ctx:
            singles = ctx.enter_context(tc.tile_pool(name="singles", bufs=1))
            xin_pool = ctx.enter_context(tc.tile_pool(name="xin", bufs=8))
            xt_pool = ctx.enter_context(tc.tile_pool(name="xt", bufs=1))
            exp_pool = ctx.enter_context(tc.tile_pool(name="exp", bufs=6))
            nrm_pool = ctx.enter_context(tc.tile_pool(name="nrm", bufs=3))
            ysb_pool = ctx.enter_context(tc.tile_pool(name="ysb", bufs=4))
            # PSUM: tag "big" 2x[128,1024] (qT/kT pairs, then score pairs),
            # "mid" 2x[128,512] (vT, then AV), "y2" 2x[128,512]
            # (transpose staging, then out-proj) = 8 banks exactly.
            ps = ctx.enter_context(tc.tile_pool(name="ps", bufs=2, space="PSUM"))

            # ---- prefetch first x block before the weights ----
            def load_block(tb):
                xin = []
                for tsub in range(TS // P):
                    xt_in = xin_pool.tile([P, D], FPR, tag="xin", name=f"xin{tb}_{tsub}")
                    nc.sync.dma_start(
                        out=xt_in, in_=x[TS * tb + P * tsub: TS * tb + P * (tsub + 1), :]
                    )
                    xin.append(xt_in)
                xt = xt_pool.tile([P, NDC * TS], FPR, tag="xt", name=f"xt{tb}")
                return xin, xt

            blk0 = load_block(0)

            # ---- weights / constants ----
            wq_sb = singles.tile([P, NDC * JC], FPR, tag="wq")   # chunk c at [JC*c, JC*(c+1))
            wk_sb = singles.tile([P, NDC * JC], FPR, tag="wk")
            wv_sb = singles.tile([P, NDC * JC], FPR, tag="wv")
            for c in range(NDC):
                nc.sync.dma_start(out=wq_sb[:, JC * c:JC * (c + 1)], in_=wq[P * c:P * (c + 1), :])
                nc.sync.dma_start(out=wk_sb[:, JC * c:JC * (c + 1)], in_=wk[P * c:P * (c + 1), :])
                nc.sync.dma_start(out=wv_sb[:, JC * c:JC * (c + 1)], in_=wv[P * c:P * (c + 1), :])
            wo_sb = [singles.tile([P, D], FPR, tag=f"wo{j}", name=f"wo_sb{j}") for j in range(2)]
            for j in range(2):
                nc.sync.dma_start(out=wo_sb[j], in_=wo[P * j:P * (j + 1), :])
            bq_sb = [singles.tile([P, 1], FP, tag=f"bq{j}", name=f"bq_sb{j}") for j in range(2)]
            bk_sb = [singles.tile([P, 1], FP, tag=f"bk{j}", name=f"bk_sb{j}") for j in range(2)]
            for j in range(2):
                nc.sync.dma_start(out=bq_sb[j], in_=bq[P * j:P * (j + 1), :])
                nc.sync.dma_start(out=bk_sb[j], in_=bk[P * j:P * (j + 1), :])

            # f32r identity for PE transposes: zero via x*0, 1.0 diagonal via
            # affine_select (memset/iota can't write f32r floats).
            ident = singles.tile([P, P], FPR)
            nc.vector.tensor_scalar(
                out=ident, in0=wq_sb[:, 0:P], scalar1=0.0, scalar2=None,
                op0=mybir.AluOpType.mult,
            )
            nc.gpsimd.affine_select(
                out=ident, in_=ident, compare_op=mybir.AluOpType.not_equal,
                fill=1.0, base=0, channel_multiplier=1, pattern=[[-1, P]],
            )

            # persistent activations
            qt_sb = [singles.tile([P, T], FPR, tag=f"qt{j}", name=f"qt_sb{j}") for j in range(2)]
            kt_sb = [singles.tile([P, T], FPR, tag=f"kt{j}", name=f"kt_sb{j}") for j in range(2)]
            ao_sb = [singles.tile([P, T], FPR, tag=f"ao{j}", name=f"ao_sb{j}") for j in range(2)]
            # v_aug: l-chunk lc at [VA*HL*lc, ...), head h at offset VA*h, ones at +DH
            n_lch = T // LCH
            vaug = singles.tile([P, n_lch * HL * VA], FPR, tag="vaug")
            vaug_g = vaug.rearrange("p (c v) -> p c v", v=VA)
            # ones columns: 1.0 = in*0 + 1 on DVE (memset rejects f32r, iota
            # writes raw int bits on HW)
            nc.vector.tensor_scalar(
                out=vaug_g[:, :, DH], in0=wq_sb[:, 0:n_lch * HL],
                scalar1=0.0, scalar2=1.0,
                op0=mybir.AluOpType.mult, op1=mybir.AluOpType.add,
            )

            # PE warm-up: dummy matmuls during the initial DMA wait get the
            # HAM clock gate to full rate before the real work arrives.
            warm = ps.tile([P, P], FP, tag="fil", name="warm")
            for _ in range(24):
                nc.tensor.matmul(warm, ident, ident, start=True, stop=True)

            # ---------- emission units (software-pipelined schedule) ----------
            def transpose_units(tb, xin, xt):
                def make(c):
                    def emit():
                        tr = ps.tile([P, TS], FPR, tag="fil", name=f"tr{tb}_{c}")
                        for tsub in range(TS // P):
                            nc.tensor.transpose(
                                tr[:, P * tsub:P * (tsub + 1)],
                                xin[tsub][:, P * c:P * (c + 1)],
                                ident,
                            )
                        if c % 2 == 0:
                            nc.vector.tensor_copy(out=xt[:, TS * c:TS * (c + 1)], in_=tr)
                        else:
                            nc.scalar.activation(
                                out=xt[:, TS * c:TS * (c + 1)], in_=tr,
                                func=mybir.ActivationFunctionType.Copy,
                            )
                    return emit
                return [make(c) for c in range(NDC)]

            def proj_units(tb, xt):
                """Single-bank filler units: q/k transposed per j-tile, v in
                natural [token, head-col] layout straight into v_aug."""
                units = []

                def make_qk(which, w_sb, out_sb, j):
                    box = [None]

                    def emit_lo():
                        box[0] = ps.tile([P, TS], FP, tag="fil", name=f"{which}p{tb}_{j}")
                        for c in range(NDC // 2):
                            nc.tensor.matmul(
                                box[0],
                                w_sb[:, JC * c + P * j:JC * c + P * (j + 1)],
                                xt[:, TS * c:TS * (c + 1)],
                                start=(c == 0), stop=False,
                            )

                    def emit_hi():
                        acc = box[0]
                        for c in range(NDC // 2, NDC):
                            nc.tensor.matmul(
                                acc,
                                w_sb[:, JC * c + P * j:JC * c + P * (j + 1)],
                                xt[:, TS * c:TS * (c + 1)],
                                start=False, stop=(c == NDC - 1),
                            )
                        if which == "qt":
                            nc.vector.tensor_scalar(
                                out=out_sb[j][:, TS * tb:TS * (tb + 1)], in0=acc,
                                scalar1=0.125, scalar2=bq_sb[j],
                                op0=mybir.AluOpType.mult, op1=mybir.AluOpType.add,
                            )
                        else:
                            nc.vector.tensor_scalar(
                                out=out_sb[j][:, TS * tb:TS * (tb + 1)], in0=acc,
                                scalar1=bk_sb[j], scalar2=None, op0=mybir.AluOpType.add,
                            )
                    return [emit_lo, emit_hi]

                def make_v(s):
                    box = [None]

                    def make_w(w):
                        def emit():
                            # natural [t, j] layout (xT chunk is the stationary);
                            # sequential accumulation groups per bank half
                            if w == 0:
                                box[0] = ps.tile([P, TS], FP, tag="fil", name=f"vp{tb}_{s}")
                            acc = box[0]
                            ts_ = 2 * s + w
                            for c in range(NDC):
                                nc.tensor.matmul(
                                    acc[:, JC * w:JC * (w + 1)],
                                    xt[:, TS * c + P * ts_:TS * c + P * (ts_ + 1)],
                                    wv_sb[:, JC * c:JC * (c + 1)],
                                    start=(c == 0), stop=(c == NDC - 1),
                                )
                            lc = 4 * tb + ts_
                            nc.vector.tensor_copy(
                                out=vaug_g[:, HL * lc:HL * (lc + 1), 0:DH],
                                in_=acc[:, JC * w:JC * (w + 1)].rearrange(
                                    "p (h d) -> p h d", d=DH
                                ),
                            )
                        return emit
                    return [make_w(0), make_w(1)]

                for j in range(2):
                    units.extend(make_qk("qt", wq_sb, qt_sb, j))
                    units.extend(make_qk("kt", wk_sb, kt_sb, j))
                for s in range(2):
                    units.extend(make_v(s))
                return units

            def attn_units(i):
                nch = 4 * (i + 1)   # causal chunks
                units = []
                for jp in range(2):          # head pair (2*jp, 2*jp+1)
                    avs = [None, None]

                    def make_pair_start(i, jp, avs):
                        def emit():
                            for u in range(2):
                                avs[u] = ps.tile(
                                    [P, IB], FP, tag="mid", name=f"av{i}_{2 * jp + u}"
                                )
                        return emit

                    def make_c(i, jp, avs, c):
                        def emit():
                            # Diagonal chunks only need query columns >= 128*v
                            # (earlier ones are fully masked). f32r needs the
                            # moving dim >= 256, so clamp the column offset.
                            diag = c >= 4 * i
                            v = c - 4 * i if diag else 0
                            off = P * min(v, 2) if diag else 0
                            ne = IB - off
                            # both heads' scoresT for chunk c in one 2-bank tile;
                            # the two matmuls occupy disjoint PE row groups
                            # (K=64 at rows 0-63 / 64-127) and run concurrently.
                            sc = ps.tile([P, 2 * IB], FP, tag="big", name=f"sc{i}_{jp}_{c}")
                            for u in range(2):
                                ro = DH * u
                                nc.tensor.matmul(
                                    sc[:, IB * u + off:IB * (u + 1)],
                                    kt_sb[jp][ro:ro + DH, LCH * c:LCH * (c + 1)],
                                    qt_sb[jp][ro:ro + DH, IB * i + off:IB * (i + 1)],
                                    start=True, stop=True,
                                )
                            ex = exp_pool.tile([P, 2 * IB], FPR, tag="ex", name=f"ex{i}_{jp}_{c}")
                            sc_g = sc.rearrange("p (u n) -> p u n", u=2)
                            ex_g = ex.rearrange("p (u n) -> p u n", u=2)
                            nc.scalar.activation(
                                out=ex_g[:, :, off:], in_=sc_g[:, :, off:],
                                func=mybir.ActivationFunctionType.Exp,
                            )
                            if diag:
                                # zero exp() where l > i over the computed cols
                                # [off, 512): keep (f_global - 128v) - p >= 0
                                for u in range(2):
                                    nc.gpsimd.affine_select(
                                        out=ex[:, IB * u + off:IB * (u + 1)],
                                        in_=ex[:, IB * u + off:IB * (u + 1)],
                                        compare_op=mybir.AluOpType.is_ge,
                                        fill=0.0, base=off - P * v,
                                        channel_multiplier=-1, pattern=[[1, ne]],
                                    )
                            for u in range(2):
                                h = 2 * jp + u
                                nc.tensor.matmul(
                                    avs[u][0:VA, off:],
                                    vaug[:, VA * HL * c + VA * h: VA * HL * c + VA * (h + 1)],
                                    ex[:, IB * u + off:IB * (u + 1)],
                                    start=(c == 0), stop=(c == nch - 1),
                                    skip_group_check=True,
                                )
                        return emit

                    def make_tail(i, jp, avs, u):
                        def emit():
                            h = 2 * jp + u
                            ro = DH * u
                            recip = nrm_pool.tile([1, IB], FP, tag="rc", name=f"rc{i}_{h}")
                            nc.vector.reciprocal(out=recip, in_=avs[u][DH:DH + 1, :])
                            bc = nrm_pool.tile([DH, IB], FP, tag="bc", name=f"bc{i}_{h}")
                            nc.gpsimd.partition_broadcast(out_ap=bc, in_ap=recip)
                            nc.vector.tensor_mul(
                                out=ao_sb[jp][ro:ro + DH, IB * i:IB * (i + 1)],
                                in0=avs[u][0:DH, :], in1=bc,
                            )
                        return emit

                    units.append(make_pair_start(i, jp, avs))
                    for c in range(nch):
                        units.append(make_c(i, jp, avs, c))
                    units.append(make_tail(i, jp, avs, 0))
                    units.append(make_tail(i, jp, avs, 1))
                return units

            def y_units(i):
                units = []

                def make(tt):
                    def emit():
                        ysb = ysb_pool.tile([P, D], FP, tag="ysb", name=f"ysb{tt}")
                        for db in range(2):
                            yps = ps.tile([P, IB], FP, tag="fil", name=f"yps{tt}_{db}")
                            for j in range(2):
                                nc.tensor.matmul(
                                    yps,
                                    ao_sb[j][:, P * tt:P * (tt + 1)],
                                    wo_sb[j][:, IB * db:IB * (db + 1)],
                                    start=(j == 0), stop=(j == 1),
                                )
                            if (tt + db) % 2 == 0:
                                nc.scalar.activation(
                                    out=ysb[:, IB * db:IB * (db + 1)], in_=yps,
                                    func=mybir.ActivationFunctionType.Copy,
                                )
                            else:
                                nc.vector.tensor_copy(
                                    out=ysb[:, IB * db:IB * (db + 1)], in_=yps,
                                )
                        nc.sync.dma_start(out=y[P * tt:P * (tt + 1), :], in_=ysb)
                    return emit
                for tt in range(4 * i, 4 * (i + 1)):
                    units.append(make(tt))
                return units

            def interleave(main, fillers):
                """Emit `main` units with `fillers` spread evenly between them."""
                if not main:
                    for f in fillers:
                        f()
                    return
                nf = len(fillers)
                fi = 0
                for k, m in enumerate(main):
                    m()
                    want = (k + 1) * nf // len(main)
                    while fi < want:
                        fillers[fi]()
                        fi += 1
                while fi < nf:
                    fillers[fi]()
                    fi += 1

            # ---------- pipelined schedule ----------
            # NOTE: Tile is a *tracing* scheduler — emission order defines the
            # dataflow. Every consumer must be emitted after its producer, so
            # block-0 setup runs as a strict prologue.
            for u in transpose_units(0, *blk0):
                u()
            for u in proj_units(0, blk0[1]):
                u()

            for tb in range(NTB):
                fillers = []
                if tb + 1 < NTB:
                    nxt = load_block(tb + 1)
                    fillers += transpose_units(tb + 1, *nxt)
                    fillers += proj_units(tb + 1, nxt[1])
                else:
                    # the last attention block is the most exp-bound and has no
                    # next-block setup to hide: park ALL deferred out-projection
                    # blocks here (slots tb=1,2 are PE-overloaded already)
                    for i_y in range(NTB - 1):
                        fillers += y_units(i_y)
                # attention for block tb is ACT(exp)-bound: fill PE gaps with
                # next block's transposes/projections and the previous block's
                # out-projection
                interleave(attn_units(tb), fillers)
            for u in y_units(NTB - 1):
                u()

    nc.compile()
    return nc


def get_nc():
    if "nc" not in _CACHE:
        _CACHE["nc"] = build_nc()
    return _CACHE["nc"]


def kernel(x, wq, bq, wk, bk, wv, bv, wo, bo):
    x = np.ascontiguousarray(np.asarray(x, dtype=np.float32))
    wq = np.asarray(wq, dtype=np.float32)
    wk = np.asarray(wk, dtype=np.float32)
    wv = np.asarray(wv, dtype=np.float32)
    wo = np.asarray(wo, dtype=np.float32)
    bq = np.asarray(bq, dtype=np.float32)
    bk = np.asarray(bk, dtype=np.float32)
    bv = np.asarray(bv, dtype=np.float32)
    bo = np.asarray(bo, dtype=np.float32)

    nc = get_nc()
    in_maps = []
    for core in range(NCORES):
        b, g = divmod(core, GROUPS)
        cs = slice(JC * g, JC * (g + 1))
        in_maps.append({
            "x": np.ascontiguousarray(x[b]),
            "wq": np.ascontiguousarray(wq[:, cs]),
            "wk": np.ascontiguousarray(wk[:, cs]),
            "wv": np.ascontiguousarray(wv[:, cs]),
            "wo": np.ascontiguousarray(wo[cs, :]),
            "bq": np.ascontiguousarray(bq[cs].reshape(JC, 1)),
            "bk": np.ascontiguousarray(bk[cs].reshape(JC, 1)),
        })
    res = run_bass_kernel_spmd(nc, in_maps, list(range(NCORES)))
    _CACHE["last_results"] = res

    out = np.zeros((B, S, D), np.float32)
    for core in range(NCORES):
        out[core // GROUPS] += res.results[core]["y"]
    # bv and bo never pass through softmax nonlinearity: rows of attn sum to 1,
    # so (v + bv) contributes exactly bv @ wo to every output row.
    out += (bv @ wo + bo)[None, None, :]
    return out



# revision 23
# speedup vs baseline: 1.2581x; 1.2581x over previous
"""Causal self-attention on 8 TRN2 NeuronCores.

Sharding: data-parallel over batch (2) x tensor-parallel over heads (4 heads
per core). Core c handles batch c//4, heads 4*(c%4)..4*(c%4)+3 — i.e. columns
[256*g, 256*(g+1)) of wq/wk/wv and rows [256*g, 256*(g+1)) of wo. Each core
returns a partial output [2048, 1024]; the host sums the 4 partials of each
batch and adds the (bv @ wo + bo) correction (exact because softmax rows sum
to 1).

Per-core kernel (Tile framework, fully unrolled, bf16 operands / fp32 psum):
  1. x arrives TRANSPOSED via DMA-XBAR transpose (bf16, 2-byte path) straight
     into xt [128, 8*2048] — no PE transposes, no psum->sbuf staging copies.
     Weights are host-cast to bf16 and land in one coalesced DMA each,
     dispatched from the ACT queue so the SP queue stays clear for x/y.
  2. qT/kT [256,2048] projected with xt as the moving operand (j on
     partitions; q scaled by 1/8 + bq, k + bk fused into the psum->sbuf
     move). v projected in natural [t, j] layout straight into v_aug, which
     carries a ones column per head ([128, 65] groups) so the AV matmul also
     produces the softmax denominator in row 64.
  3. Attention per (head-pair, 512-wide i-block), scores kept TRANSPOSED
     ([l-chunk=128, i=512]). Causal: chunks above the diagonal are skipped,
     diagonal chunks compute only the live column range [128v, 512) (bf16 has
     no moving>=256 constraint) and only the [128,128] triangle block gets
     exp() zeroed via gpsimd.affine_select. Normalization: DVE reciprocal of
     psum row 64, gpsimd partition_broadcast, DVE multiply.
  4. y = attn_outT.T @ wo accumulated over the 2 local j-chunks, per
     128-token tile, DMA'd out (fp32).
  5. Schedule: warmup matmuls run off a memset tile (no DMA dependency) to
     ramp the PE while the first x chunks land; block-0 projections consume
     xT chunks in arrival (c-major) order; attention for block i is
     ACT(exp)-bound locally, so the next block's projections and previous
     blocks' out-projections are interleaved as filler units.
"""

import sys

import numpy as np

if "/opt/trn_rl_repo" not in sys.path:
    sys.path.insert(0, "/opt/trn_rl_repo")

import concourse.mybir as mybir
import concourse.tile as tile
from concourse import bacc
from concourse.bass_utils import run_bass_kernel_spmd

# Problem shapes (hardcoded per contract)
B, S, D = 2, 2048, 1024
H, DH = 16, 64
NCORES = 8
GROUPS = 4                  # tensor-parallel groups per batch
HL = H // GROUPS            # 4 local heads
JC = HL * DH                # 256 local head columns
T = S                       # tokens per core (one batch element)

P = 128                     # partitions
TS = 512                    # token block (projection granularity)
NTB = T // TS               # 4 token blocks
NDC = D // P                # 8 contraction chunks
IB = 512                    # attention i-block (query positions)
LCH = P                     # attention l-chunk (key positions)
VA = DH + 1                 # v_aug columns per head (ones column appended)

FP = mybir.dt.float32
BF = mybir.dt.bfloat16

_CACHE = {}


def build_nc():
    nc = bacc.Bacc("TRN2", target_bir_lowering=False, debug=False)

    # x arrives HOST-TRANSPOSED as [D, T] so every load is a plain DMACopy:
    # mixing XBAR-transpose DMAs with normal copies on the HWDGE ring forces
    # a full completion chain at every mode switch.
    x = nc.dram_tensor("x", [D, T], BF, kind="ExternalInput")
    wq = nc.dram_tensor("wq", [D, JC], BF, kind="ExternalInput")
    wk = nc.dram_tensor("wk", [D, JC], BF, kind="ExternalInput")
    wv = nc.dram_tensor("wv", [D, JC], BF, kind="ExternalInput")
    wo = nc.dram_tensor("wo", [JC, D], BF, kind="ExternalInput")
    bq = nc.dram_tensor("bq", [P, 2], FP, kind="ExternalInput")   # host-pretiled
    bk = nc.dram_tensor("bk", [P, 2], FP, kind="ExternalInput")
    y = nc.dram_tensor("y", [T, D], FP, kind="ExternalOutput")

    with tile.TileContext(nc) as tc:
        import contextlib

        with contextlib.ExitStack() as ctx:
            singles = ctx.enter_context(tc.tile_pool(name="singles", bufs=1))
            exp_pool = ctx.enter_context(tc.tile_pool(name="exp", bufs=6))
            nrm_pool = ctx.enter_context(tc.tile_pool(name="nrm", bufs=3))
            ysb_pool = ctx.enter_context(tc.tile_pool(name="ysb", bufs=4))
            # PSUM: tag "big" 2x[128,1024] (block-0 q accs, then score pairs),
            # "mid" 2x[128,512] (block-0 k accs, then AV), "fil" 2x[128,512]
            # (proj/y accumulators) = 8 banks exactly.
            ps = ctx.enter_context(tc.tile_pool(name="ps", bufs=2, space="PSUM"))

            # ---- warmup source: no DMA dependency, Pool memset at t=0 ----
            wsrc = singles.tile([P, P], BF, tag="wsrc")
            nc.gpsimd.memset(wsrc, 0.0)
            warm = ps.tile([P, P], FP, tag="fil", name="warm")
            for _ in range(28):
                nc.tensor.matmul(warm, wsrc, wsrc, start=True, stop=True)

            # ---- persistent tiles ----
            # xt chunk c at [T*c, T*(c+1)); token block tb at offset TS*tb
            xt = singles.tile([P, NDC * T], BF, tag="xt")
            wq_sb = singles.tile([P, NDC * JC], BF, tag="wq")   # chunk c at [JC*c, JC*(c+1))
            wk_sb = singles.tile([P, NDC * JC], BF, tag="wk")
            wv_sb = singles.tile([P, NDC * JC], BF, tag="wv")
            wo_sb = singles.tile([P, 2 * D], BF, tag="wo")      # j at [D*j, D*(j+1))
            bq_sb = singles.tile([P, 2], FP, tag="bq")
            bk_sb = singles.tile([P, 2], FP, tag="bk")
            qt_sb = [singles.tile([P, T], BF, tag=f"qt{j}", name=f"qt_sb{j}") for j in range(2)]
            kt_sb = [singles.tile([P, T], BF, tag=f"kt{j}", name=f"kt_sb{j}") for j in range(2)]
            ao_sb = [singles.tile([P, T], BF, tag=f"ao{j}", name=f"ao_sb{j}") for j in range(2)]
            # v_aug: l-chunk lc at [VA*HL*lc, ...), head h at offset VA*h, ones at +DH
            n_lch = T // LCH
            vaug = singles.tile([P, n_lch * HL * VA], BF, tag="vaug")
            vaug_g = vaug.rearrange("p (c v) -> p c v", v=VA)

            # ---- DMA dispatch ----
            # All plain DMACopies on the single SP queue, in consumption
            # order: the 8-slot HWDGE ring pipelines freely when neither the
            # queue nor the DMA type changes between entries.
            def xt0_dma(c):
                nc.sync.dma_start(
                    out=xt[:, T * c: T * c + TS],
                    in_=x[P * c: P * (c + 1), 0:TS],
                )

            def w_dma(w_sb, w_dram):
                nc.sync.dma_start(
                    out=w_sb.rearrange("p (c j) -> p c j", j=JC),
                    in_=w_dram.rearrange("(c p) j -> p c j", p=P),
                )

            w_dma(wq_sb, wq)
            xt0_dma(0)
            xt0_dma(1)
            w_dma(wk_sb, wk)
            xt0_dma(2)
            xt0_dma(3)
            xt0_dma(4)
            nc.sync.dma_start(out=bq_sb, in_=bq[:, :])
            nc.sync.dma_start(out=bk_sb, in_=bk[:, :])
            xt0_dma(5)
            xt0_dma(6)
            xt0_dma(7)
            w_dma(wv_sb, wv)
            # x token-blocks 1-3 (one [128,1536] DMA per chunk)
            for c in range(NDC):
                nc.sync.dma_start(
                    out=xt[:, T * c + TS: T * (c + 1)],
                    in_=x[P * c: P * (c + 1), TS:T],
                )
            # wo last: not needed until the first out-projection (~40us)
            nc.sync.dma_start(
                out=wo_sb.rearrange("p (j d) -> p j d", d=D),
                in_=wo.rearrange("(j p) d -> p j d", p=P),
            )

            # ones columns of v_aug: 1.0 = wsrc*0 + 1 on DVE (no DMA dep)
            nc.vector.tensor_scalar(
                out=vaug_g[:, :, DH], in0=wsrc[:, 0:n_lch * HL],
                scalar1=0.0, scalar2=1.0,
                op0=mybir.AluOpType.mult, op1=mybir.AluOpType.add,
            )
            # Causal-mask constants: scores[tri] += (negI.T @ U)[l,q'] =
            # -3e4 * [q' < l], folded into the score matmul so no gpsimd
            # affine_select sits in the score->exp->AV chain.
            negi = singles.tile([P, P], BF, tag="negi")
            nc.gpsimd.memset(negi, 0.0)
            nc.gpsimd.affine_select(
                out=negi, in_=negi, compare_op=mybir.AluOpType.not_equal,
                fill=-30000.0, base=0, channel_multiplier=1, pattern=[[-1, P]],
            )
            utri = singles.tile([P, P], BF, tag="utri")
            nc.gpsimd.memset(utri, 0.0)
            # keep 0 where q' - l + 1 > 0 (q' >= l), fill 1 where q' < l
            nc.gpsimd.affine_select(
                out=utri, in_=utri, compare_op=mybir.AluOpType.is_gt,
                fill=1.0, base=1, channel_multiplier=-1, pattern=[[1, P]],
            )

            # ---------- emission units ----------
            def qk_bias(which, j, acc, tb):
                if which == "qt":
                    nc.vector.tensor_scalar(
                        out=qt_sb[j][:, TS * tb:TS * (tb + 1)], in0=acc,
                        scalar1=0.125, scalar2=bq_sb[:, j:j + 1],
                        op0=mybir.AluOpType.mult, op1=mybir.AluOpType.add,
                    )
                else:
                    nc.vector.tensor_scalar(
                        out=kt_sb[j][:, TS * tb:TS * (tb + 1)], in0=acc,
                        scalar1=bk_sb[:, j:j + 1], scalar2=None,
                        op0=mybir.AluOpType.add,
                    )

            def proj0_units():
                """Block-0 projections in chunk-major order so each xT chunk
                is consumed 4x as it lands. q accs on 'big', k accs on 'mid'
                (attention hasn't started; those banks are free)."""
                accs = {}

                def start_unit():
                    accs["q0"] = ps.tile([P, TS], FP, tag="big", name="p0q0")
                    accs["q1"] = ps.tile([P, TS], FP, tag="big", name="p0q1")
                    accs["k0"] = ps.tile([P, TS], FP, tag="mid", name="p0k0")
                    accs["k1"] = ps.tile([P, TS], FP, tag="mid", name="p0k1")

                def make_c(c):
                    def emit():
                        for which, w_sb in (("q", wq_sb), ("k", wk_sb)):
                            for j in range(2):
                                nc.tensor.matmul(
                                    accs[f"{which}{j}"],
                                    w_sb[:, JC * c + P * j:JC * c + P * (j + 1)],
                                    xt[:, T * c:T * c + TS],
                                    start=(c == 0), stop=(c == NDC - 1),
                                )
                    return emit

                def finish_unit():
                    qk_bias("qt", 0, accs["q0"], 0)
                    qk_bias("qt", 1, accs["q1"], 0)
                    qk_bias("kt", 0, accs["k0"], 0)
                    qk_bias("kt", 1, accs["k1"], 0)

                units = [start_unit]
                units += [make_c(c) for c in range(NDC)]
                units.append(finish_unit)
                return units

            def qk_units(tb):
                """Filler-grade q/k projection for block tb>=1: one 'fil' psum
                held across lo+hi halves per (which, j)."""
                units = []

                def make(which, w_sb, j):
                    box = [None]

                    def emit_lo():
                        box[0] = ps.tile([P, TS], FP, tag="fil", name=f"{which}p{tb}_{j}")
                        for c in range(NDC // 2):
                            nc.tensor.matmul(
                                box[0],
                                w_sb[:, JC * c + P * j:JC * c + P * (j + 1)],
                                xt[:, T * c + TS * tb:T * c + TS * (tb + 1)],
                                start=(c == 0), stop=False,
                            )

                    def emit_hi():
                        for c in range(NDC // 2, NDC):
                            nc.tensor.matmul(
                                box[0],
                                w_sb[:, JC * c + P * j:JC * c + P * (j + 1)],
                                xt[:, T * c + TS * tb:T * c + TS * (tb + 1)],
                                start=False, stop=(c == NDC - 1),
                            )
                        qk_bias(which, j, box[0], tb)
                    return [emit_lo, emit_hi]

                for j in range(2):
                    units += make("qt", wq_sb, j)
                    units += make("kt", wk_sb, j)
                return units

            def v_units(tb):
                """v in natural [t, j] layout straight into v_aug."""
                units = []

                def make(s):
                    box = [None]

                    def make_w(w):
                        def emit():
                            if w == 0:
                                box[0] = ps.tile([P, TS], FP, tag="fil", name=f"vp{tb}_{s}")
                            acc = box[0]
                            ts_ = 2 * s + w
                            for c in range(NDC):
                                nc.tensor.matmul(
                                    acc[:, JC * w:JC * (w + 1)],
                                    xt[:, T * c + TS * tb + P * ts_:T * c + TS * tb + P * (ts_ + 1)],
                                    wv_sb[:, JC * c:JC * (c + 1)],
                                    start=(c == 0), stop=(c == NDC - 1),
                                )
                            lc = 4 * tb + ts_
                            # gpsimd cannot read PSUM on real HW
                            nc.vector.tensor_copy(
                                out=vaug_g[:, HL * lc:HL * (lc + 1), 0:DH],
                                in_=acc[:, JC * w:JC * (w + 1)].rearrange(
                                    "p (h d) -> p h d", d=DH
                                ),
                            )
                        return emit
                    return [make_w(0), make_w(1)]

                for s in range(2):
                    units += make(s)
                return units

            def proj_units(tb):
                return qk_units(tb) + v_units(tb)

            def attn_units(i, jp_order=(0, 1)):
                nch = 4 * (i + 1)   # causal chunks
                units = []
                for jp in jp_order:          # head pair (2*jp, 2*jp+1)
                    avs = [None, None]

                    def make_pair_start(i, jp, avs):
                        def emit():
                            for u in range(2):
                                avs[u] = ps.tile(
                                    [P, IB], FP, tag="mid", name=f"av{i}_{2 * jp + u}"
                                )
                        return emit

                    def make_c(i, jp, avs, c):
                        def emit():
                            # Diagonal chunk v: only columns >= 128*v are live;
                            # the [128,128] block at [128v, 128(v+1)) is
                            # triangular, everything right of it fully live.
                            diag = c >= 4 * i
                            v = c - 4 * i if diag else 0
                            off = P * v if diag else 0
                            sc = ps.tile([P, 2 * IB], FP, tag="big", name=f"sc{i}_{jp}_{c}")
                            for u in range(2):
                                ro = DH * u
                                nc.tensor.matmul(
                                    sc[:, IB * u + off:IB * (u + 1)],
                                    kt_sb[jp][ro:ro + DH, LCH * c:LCH * (c + 1)],
                                    qt_sb[jp][ro:ro + DH, IB * i + off:IB * (i + 1)],
                                    start=True, stop=not diag,
                                    skip_group_check=True,
                                )
                                if diag:
                                    # scores[tri] += -3e4 where q' < l: masked
                                    # exp underflows to 0, no affine_select in
                                    # the chain
                                    nc.tensor.matmul(
                                        sc[:, IB * u + off:IB * u + off + P],
                                        negi, utri,
                                        start=False, stop=True,
                                        skip_group_check=True,
                                    )
                            ex = exp_pool.tile([P, 2 * IB], BF, tag="ex", name=f"ex{i}_{jp}_{c}")
                            sc_g = sc.rearrange("p (u n) -> p u n", u=2)
                            ex_g = ex.rearrange("p (u n) -> p u n", u=2)
                            nc.scalar.activation(
                                out=ex_g[:, :, off:], in_=sc_g[:, :, off:],
                                func=mybir.ActivationFunctionType.Exp,
                            )
                            for u in range(2):
                                h = 2 * jp + u
                                nc.tensor.matmul(
                                    avs[u][0:VA, off:],
                                    vaug[:, VA * (HL * c + h): VA * (HL * c + h + 1)],
                                    ex[:, IB * u + off:IB * (u + 1)],
                                    start=(c == 0), stop=(c == nch - 1),
                                    skip_group_check=True,
                                )
                        return emit

                    def make_tail(i, jp, avs, u, w):
                        """Normalize the 128-query column slice w of i-block i:
                        avs columns [128w, 128(w+1)) are final as soon as diag
                        chunk v=w has accumulated, so these chains hide under
                        the remaining diagonal chunks instead of trailing the
                        whole block."""
                        def emit():
                            h = 2 * jp + u
                            ro = DH * u
                            cw = P * w
                            recip = nrm_pool.tile([1, P], FP, tag="rc", name=f"rc{i}_{h}_{w}")
                            nc.vector.reciprocal(
                                out=recip, in_=avs[u][DH:DH + 1, cw:cw + P]
                            )
                            bc = nrm_pool.tile([DH, P], FP, tag="bc", name=f"bc{i}_{h}_{w}")
                            nc.gpsimd.partition_broadcast(out_ap=bc, in_ap=recip)
                            nc.vector.tensor_mul(
                                out=ao_sb[jp][ro:ro + DH, IB * i + cw:IB * i + cw + P],
                                in0=avs[u][0:DH, cw:cw + P], in1=bc,
                            )
                        return emit

                    units.append(make_pair_start(i, jp, avs))
                    for c in range(nch):
                        units.append(make_c(i, jp, avs, c))
                        if c >= 4 * i:
                            w = c - 4 * i
                            units.append(make_tail(i, jp, avs, 0, w))
                            units.append(make_tail(i, jp, avs, 1, w))
                return units

            def y_units(i, j_order=(0, 1), split_dma=False):
                units = []

                def make(tt):
                    def emit():
                        ysb = ysb_pool.tile([P, D], FP, tag="ysb", name=f"ysb{tt}")
                        for db in range(2):
                            yps = ps.tile([P, IB], FP, tag="fil", name=f"yps{tt}_{db}")
                            for k, j in enumerate(j_order):
                                nc.tensor.matmul(
                                    yps,
                                    ao_sb[j][:, P * tt:P * (tt + 1)],
                                    wo_sb[:, D * j + IB * db:D * j + IB * (db + 1)],
                                    start=(k == 0), stop=(k == 1),
                                )
                            # gpsimd cannot read PSUM on real HW: split the
                            # psum->sbuf copies between ACT and DVE. Final
                            # tail tiles all go to ACT (idle after last exp).
                            if (split_dma and tt % 4 >= 2) or (tt + db) % 2 == 0:
                                nc.scalar.activation(
                                    out=ysb[:, IB * db:IB * (db + 1)], in_=yps,
                                    func=mybir.ActivationFunctionType.Copy,
                                )
                            else:
                                nc.vector.tensor_copy(
                                    out=ysb[:, IB * db:IB * (db + 1)], in_=yps,
                                )
                            if split_dma:
                                nc.sync.dma_start(
                                    out=y[P * tt:P * (tt + 1), IB * db:IB * (db + 1)],
                                    in_=ysb[:, IB * db:IB * (db + 1)],
                                )
                        if not split_dma:
                            nc.sync.dma_start(out=y[P * tt:P * (tt + 1), :], in_=ysb)
                    return emit
                for tt in range(4 * i, 4 * (i + 1)):
                    units.append(make(tt))
                return units

            def interleave(main, fillers):
                """Emit `main` units with `fillers` spread evenly between them."""
                if not main:
                    for f in fillers:
                        f()
                    return
                nf = len(fillers)
                fi = 0
                for k, m in enumerate(main):
                    m()
                    want = (k + 1) * nf // len(main)
                    while fi < want:
                        fillers[fi]()
                        fi += 1
                while fi < nf:
                    fillers[fi]()
                    fi += 1

            # ---------- pipelined schedule ----------
            # NOTE: Tile is a *tracing* scheduler — emission order defines the
            # dataflow. Every consumer must be emitted after its producer, so
            # block-0 setup runs as a strict prologue.
            for u in proj0_units():
                u()
            for u in v_units(0):
                u()

            for tb in range(NTB):
                fillers = []
                if tb + 1 < NTB:
                    fillers += proj_units(tb + 1)
                # y(0) into block 1; y(1) and y(2) into block 3 (block 2 is
                # already PE-bound with proj(3); block 3 has the most
                # ACT-bound slack to fill)
                if tb == 1:
                    fillers += y_units(0)
                elif tb == 3:
                    fillers += y_units(1) + y_units(2)
                # last block: run jp=1 first so ao_sb[1] is ready early and
                # the final y accumulation (j=1 first) can start before jp=0's
                # normalize tail completes
                jp_order = (1, 0) if tb == NTB - 1 else (0, 1)
                interleave(attn_units(tb, jp_order), fillers)
            for u in y_units(NTB - 1, j_order=(1, 0), split_dma=True):
                u()

    nc.compile()
    return nc


def get_nc():
    if "nc" not in _CACHE:
        _CACHE["nc"] = build_nc()
    return _CACHE["nc"]


def kernel(x, wq, bq, wk, bk, wv, bv, wo, bo):
    import ml_dtypes
    bf16 = ml_dtypes.bfloat16

    x = np.asarray(x, dtype=np.float32)
    wq = np.asarray(wq, dtype=np.float32)
    wk = np.asarray(wk, dtype=np.float32)
    wv = np.asarray(wv, dtype=np.float32)
    wo = np.asarray(wo, dtype=np.float32)
    bq = np.asarray(bq, dtype=np.float32)
    bk = np.asarray(bk, dtype=np.float32)
    bv = np.asarray(bv, dtype=np.float32)
    bo = np.asarray(bo, dtype=np.float32)

    nc = get_nc()
    in_maps = []
    for core in range(NCORES):
        b, g = divmod(core, GROUPS)
        cs = slice(JC * g, JC * (g + 1))
        in_maps.append({
            "x": np.ascontiguousarray(x[b].astype(bf16).T),
            "wq": np.ascontiguousarray(wq[:, cs].astype(bf16)),
            "wk": np.ascontiguousarray(wk[:, cs].astype(bf16)),
            "wv": np.ascontiguousarray(wv[:, cs].astype(bf16)),
            "wo": np.ascontiguousarray(wo[cs, :].astype(bf16)),
            "bq": np.ascontiguousarray(bq[cs].reshape(2, P).T),
            "bk": np.ascontiguousarray(bk[cs].reshape(2, P).T),
        })
    res = run_bass_kernel_spmd(nc, in_maps, list(range(NCORES)))
    _CACHE["last_results"] = res

    out = np.zeros((B, S, D), np.float32)
    for core in range(NCORES):
        out[core // GROUPS] += res.results[core]["y"]
    # bv and bo never pass through softmax nonlinearity: rows of attn sum to 1,
    # so (v + bv) contributes exactly bv @ wo to every output row.
    out += (bv @ wo + bo)[None, None, :]
    return out


# revision 51
# speedup vs baseline: 1.2689x; 1.0086x over previous
"""Causal self-attention on 8 TRN2 NeuronCores.

Sharding: data-parallel over batch (2) x tensor-parallel over heads (4 heads
per core). Core c handles batch c//4, heads 4*(c%4)..4*(c%4)+3 — i.e. columns
[256*g, 256*(g+1)) of wq/wk/wv and rows [256*g, 256*(g+1)) of wo. Each core
returns a partial output [2048, 1024]; the host sums the 4 partials of each
batch and adds the (bv @ wo + bo) correction (exact because softmax rows sum
to 1).

Per-core kernel (Tile framework, fully unrolled, bf16 operands / fp32 psum):
  1. x arrives TRANSPOSED via DMA-XBAR transpose (bf16, 2-byte path) straight
     into xt [128, 8*2048] — no PE transposes, no psum->sbuf staging copies.
     Weights are host-cast to bf16 and land in one coalesced DMA each,
     dispatched from the ACT queue so the SP queue stays clear for x/y.
  2. qT/kT [256,2048] projected with xt as the moving operand (j on
     partitions; q scaled by 1/8 + bq, k + bk fused into the psum->sbuf
     move). v projected in natural [t, j] layout straight into v_aug, which
     carries a ones column per head ([128, 65] groups) so the AV matmul also
     produces the softmax denominator in row 64.
  3. Attention per (head-pair, 512-wide i-block), scores kept TRANSPOSED
     ([l-chunk=128, i=512]). Causal: chunks above the diagonal are skipped,
     diagonal chunks compute only the live column range [128v, 512) (bf16 has
     no moving>=256 constraint) and only the [128,128] triangle block gets
     exp() zeroed via gpsimd.affine_select. Normalization: DVE reciprocal of
     psum row 64, gpsimd partition_broadcast, DVE multiply.
  4. y = attn_outT.T @ wo accumulated over the 2 local j-chunks, per
     128-token tile, DMA'd out (fp32).
  5. Schedule: warmup matmuls run off a memset tile (no DMA dependency) to
     ramp the PE while the first x chunks land; block-0 projections consume
     xT chunks in arrival (c-major) order; attention for block i is
     ACT(exp)-bound locally, so the next block's projections and previous
     blocks' out-projections are interleaved as filler units.
"""

import sys

import numpy as np

if "/opt/trn_rl_repo" not in sys.path:
    sys.path.insert(0, "/opt/trn_rl_repo")

import concourse.mybir as mybir
import concourse.tile as tile
from concourse import bacc
from concourse.bass_utils import run_bass_kernel_spmd

# Problem shapes (hardcoded per contract)
B, S, D = 2, 2048, 1024
H, DH = 16, 64
NCORES = 8
GROUPS = 4                  # tensor-parallel groups per batch
HL = H // GROUPS            # 4 local heads
JC = HL * DH                # 256 local head columns
T = S                       # tokens per core (one batch element)

P = 128                     # partitions
TS = 512                    # token block (projection granularity)
NTB = T // TS               # 4 token blocks
NDC = D // P                # 8 contraction chunks
IB = 512                    # attention i-block (query positions)
LCH = P                     # attention l-chunk (key positions)
VA = DH + 1                 # v_aug columns per head (ones column appended)

FP = mybir.dt.float32
BF = mybir.dt.bfloat16

_CACHE = {}


def build_nc():
    nc = bacc.Bacc("TRN2", target_bir_lowering=False, debug=False)

    # x arrives HOST-TRANSPOSED as [D, T] so every load is a plain DMACopy:
    # mixing XBAR-transpose DMAs with normal copies on the HWDGE ring forces
    # a full completion chain at every mode switch.
    x = nc.dram_tensor("x", [D, T], BF, kind="ExternalInput")
    wq = nc.dram_tensor("wq", [D, JC], BF, kind="ExternalInput")
    wk = nc.dram_tensor("wk", [D, JC], BF, kind="ExternalInput")
    wv = nc.dram_tensor("wv", [D, JC], BF, kind="ExternalInput")
    wo = nc.dram_tensor("wo", [JC, D], BF, kind="ExternalInput")
    bq = nc.dram_tensor("bq", [P, 2], FP, kind="ExternalInput")   # host-pretiled
    bk = nc.dram_tensor("bk", [P, 2], FP, kind="ExternalInput")
    # y in bf16: halves the writeback DMA time; the host upcasts and sums the
    # four partials in fp32 (costs ~3e-3 extra rel error, budget is 2e-2)
    y = nc.dram_tensor("y", [T, D], BF, kind="ExternalOutput")

    with tile.TileContext(nc) as tc:
        import contextlib

        with contextlib.ExitStack() as ctx:
            singles = ctx.enter_context(tc.tile_pool(name="singles", bufs=1))
            exp_pool = ctx.enter_context(tc.tile_pool(name="exp", bufs=48))
            nrm_pool = ctx.enter_context(tc.tile_pool(name="nrm", bufs=8))
            ysb_pool = ctx.enter_context(tc.tile_pool(name="ysb", bufs=8))
            # PSUM: tag "big" 2x[128,1024] (block-0 q accs, then score pairs),
            # "mid" 2x[128,512] (block-0 k accs, then AV), "fil" 2x[128,512]
            # (proj/y accumulators) = 8 banks exactly.
            ps = ctx.enter_context(tc.tile_pool(name="ps", bufs=2, space="PSUM"))

            # ---- warmup source: no DMA dependency, Pool memset at t=0 ----
            wsrc = singles.tile([P, P], BF, tag="wsrc")
            nc.gpsimd.memset(wsrc, 0.0)
            warm = ps.tile([P, P], FP, tag="fil", name="warm")
            for _ in range(24):
                nc.tensor.matmul(warm, wsrc, wsrc, start=True, stop=True)

            # ---- persistent tiles ----
            # xt chunk c at [T*c, T*(c+1)); token block tb at offset TS*tb
            xt = singles.tile([P, NDC * T], BF, tag="xt")
            wq_sb = singles.tile([P, NDC * JC], BF, tag="wq")   # chunk c at [JC*c, JC*(c+1))
            wk_sb = singles.tile([P, NDC * JC], BF, tag="wk")
            wv_sb = singles.tile([P, NDC * JC], BF, tag="wv")
            wo_sb = singles.tile([P, 2 * D], BF, tag="wo")      # j at [D*j, D*(j+1))
            bq_sb = singles.tile([P, 2], FP, tag="bq")
            bk_sb = singles.tile([P, 2], FP, tag="bk")
            qt_sb = [singles.tile([P, T], BF, tag=f"qt{j}", name=f"qt_sb{j}") for j in range(2)]
            kt_sb = [singles.tile([P, T], BF, tag=f"kt{j}", name=f"kt_sb{j}") for j in range(2)]
            ao_sb = [singles.tile([P, T], BF, tag=f"ao{j}", name=f"ao_sb{j}") for j in range(2)]
            # v_aug: l-chunk lc at [VA*HL*lc, ...), head h at offset VA*h, ones at +DH
            n_lch = T // LCH
            vaug = singles.tile([P, n_lch * HL * VA], BF, tag="vaug")
            vaug_g = vaug.rearrange("p (c v) -> p c v", v=VA)

            # ---- DMA dispatch ----
            # All plain DMACopies on the single SP queue, in consumption
            # order: the 8-slot HWDGE ring pipelines freely when neither the
            # queue nor the DMA type changes between entries.
            def xt0_dma(c):
                nc.sync.dma_start(
                    out=xt[:, T * c: T * c + TS],
                    in_=x[P * c: P * (c + 1), 0:TS],
                )

            def w_dma(w_sb, w_dram):
                nc.sync.dma_start(
                    out=w_sb.rearrange("p (c j) -> p c j", j=JC),
                    in_=w_dram.rearrange("(c p) j -> p c j", p=P),
                )

            def w_half(w_sb, w_dram, h):
                nc.sync.dma_start(
                    out=w_sb.rearrange("p (c j) -> p c j", j=JC)[:, 4 * h:4 * (h + 1), :],
                    in_=w_dram.rearrange("(c p) j -> p c j", p=P)[:, 4 * h:4 * (h + 1), :],
                )

            w_half(wq_sb, wq, 0)
            xt0_dma(0)
            w_half(wq_sb, wq, 1)
            xt0_dma(1)
            w_dma(wk_sb, wk)
            xt0_dma(2)
            xt0_dma(3)
            xt0_dma(4)
            nc.sync.dma_start(out=bq_sb, in_=bq[:, :])
            nc.sync.dma_start(out=bk_sb, in_=bk[:, :])
            xt0_dma(5)
            xt0_dma(6)
            xt0_dma(7)
            w_dma(wv_sb, wv)
            # x token-blocks 1-3 (one [128,1536] DMA per chunk)
            for c in range(NDC):
                nc.sync.dma_start(
                    out=xt[:, T * c + TS: T * (c + 1)],
                    in_=x[P * c: P * (c + 1), TS:T],
                )
            # wo last: not needed until the first out-projection (~40us)
            nc.sync.dma_start(
                out=wo_sb.rearrange("p (j d) -> p j d", d=D),
                in_=wo.rearrange("(j p) d -> p j d", p=P),
            )

            # ones columns of v_aug: 1.0 = wsrc*0 + 1 on DVE (no DMA dep)
            nc.vector.tensor_scalar(
                out=vaug_g[:, :, DH], in0=wsrc[:, 0:n_lch * HL],
                scalar1=0.0, scalar2=1.0,
                op0=mybir.AluOpType.mult, op1=mybir.AluOpType.add,
            )
            # Causal-mask constants: scores[tri] += (negI.T @ U)[l,q'] =
            # -3e4 * [q' < l], folded into the score matmul so no gpsimd
            # affine_select sits in the score->exp->AV chain.
            negi = singles.tile([P, P], BF, tag="negi")
            nc.gpsimd.memset(negi, 0.0)
            nc.gpsimd.affine_select(
                out=negi, in_=negi, compare_op=mybir.AluOpType.not_equal,
                fill=-30000.0, base=0, channel_multiplier=1, pattern=[[-1, P]],
            )
            utri = singles.tile([P, P], BF, tag="utri")
            nc.gpsimd.memset(utri, 0.0)
            # keep 0 where q' - l + 1 > 0 (q' >= l), fill 1 where q' < l
            nc.gpsimd.affine_select(
                out=utri, in_=utri, compare_op=mybir.AluOpType.is_gt,
                fill=1.0, base=1, channel_multiplier=-1, pattern=[[1, P]],
            )

            # ---------- emission units ----------
            def qk_bias(which, j, acc, tb):
                if which == "qt":
                    nc.vector.tensor_scalar(
                        out=qt_sb[j][:, TS * tb:TS * (tb + 1)], in0=acc,
                        scalar1=0.125, scalar2=bq_sb[:, j:j + 1],
                        op0=mybir.AluOpType.mult, op1=mybir.AluOpType.add,
                    )
                else:
                    nc.vector.tensor_scalar(
                        out=kt_sb[j][:, TS * tb:TS * (tb + 1)], in0=acc,
                        scalar1=bk_sb[:, j:j + 1], scalar2=None,
                        op0=mybir.AluOpType.add,
                    )

            def proj0_units():
                """Block-0 projections in chunk-major order so each xT chunk
                is consumed 4x as it lands. q accs on 'big', k accs on 'mid'
                (attention hasn't started; those banks are free)."""
                accs = {}

                def start_unit():
                    accs["q0"] = ps.tile([P, TS], FP, tag="big", name="p0q0")
                    accs["q1"] = ps.tile([P, TS], FP, tag="big", name="p0q1")
                    accs["k0"] = ps.tile([P, TS], FP, tag="mid", name="p0k0")
                    accs["k1"] = ps.tile([P, TS], FP, tag="mid", name="p0k1")

                def make_c(c):
                    def emit():
                        for which, w_sb in (("q", wq_sb), ("k", wk_sb)):
                            for j in range(2):
                                nc.tensor.matmul(
                                    accs[f"{which}{j}"],
                                    w_sb[:, JC * c + P * j:JC * c + P * (j + 1)],
                                    xt[:, T * c:T * c + TS],
                                    start=(c == 0), stop=(c == NDC - 1),
                                )
                    return emit

                def finish_unit():
                    qk_bias("qt", 0, accs["q0"], 0)
                    qk_bias("qt", 1, accs["q1"], 0)
                    qk_bias("kt", 0, accs["k0"], 0)
                    qk_bias("kt", 1, accs["k1"], 0)

                units = [start_unit]
                units += [make_c(c) for c in range(NDC)]
                units.append(finish_unit)
                return units

            def qk_units(tb):
                """Filler-grade q/k projection for block tb>=1: one 'fil' psum
                held across lo+hi halves per (which, j)."""
                units = []

                def make(which, w_sb, j):
                    box = [None]

                    def emit_lo():
                        box[0] = ps.tile([P, TS], FP, tag="fil", name=f"{which}p{tb}_{j}")
                        for c in range(NDC // 2):
                            nc.tensor.matmul(
                                box[0],
                                w_sb[:, JC * c + P * j:JC * c + P * (j + 1)],
                                xt[:, T * c + TS * tb:T * c + TS * (tb + 1)],
                                start=(c == 0), stop=False,
                            )

                    def emit_hi():
                        for c in range(NDC // 2, NDC):
                            nc.tensor.matmul(
                                box[0],
                                w_sb[:, JC * c + P * j:JC * c + P * (j + 1)],
                                xt[:, T * c + TS * tb:T * c + TS * (tb + 1)],
                                start=False, stop=(c == NDC - 1),
                            )
                        qk_bias(which, j, box[0], tb)
                    return [emit_lo, emit_hi]

                for j in range(2):
                    units += make("qt", wq_sb, j)
                    units += make("kt", wk_sb, j)
                return units

            def v_units(tb):
                """v in natural [t, j] layout straight into v_aug."""
                units = []

                def make(s):
                    box = [None]

                    def make_w(w):
                        def emit():
                            if w == 0:
                                box[0] = ps.tile([P, TS], FP, tag="fil", name=f"vp{tb}_{s}")
                            acc = box[0]
                            ts_ = 2 * s + w
                            for c in range(NDC):
                                nc.tensor.matmul(
                                    acc[:, JC * w:JC * (w + 1)],
                                    xt[:, T * c + TS * tb + P * ts_:T * c + TS * tb + P * (ts_ + 1)],
                                    wv_sb[:, JC * c:JC * (c + 1)],
                                    start=(c == 0), stop=(c == NDC - 1),
                                )
                            lc = 4 * tb + ts_
                            # gpsimd cannot read PSUM on real HW
                            nc.vector.tensor_copy(
                                out=vaug_g[:, HL * lc:HL * (lc + 1), 0:DH],
                                in_=acc[:, JC * w:JC * (w + 1)].rearrange(
                                    "p (h d) -> p h d", d=DH
                                ),
                            )
                        return emit
                    return [make_w(0), make_w(1)]

                for s in range(2):
                    units += make(s)
                return units

            def proj_units(tb):
                return qk_units(tb) + v_units(tb)

            def attn_units(i, jp_order=(0, 1), after_slice=None):
                nch = 4 * (i + 1)   # causal chunks
                units = []
                for jp in jp_order:          # head pair (2*jp, 2*jp+1)
                    avs = [None, None]

                    def make_pair_start(i, jp, avs):
                        def emit():
                            for u in range(2):
                                avs[u] = ps.tile(
                                    [P, IB], FP, tag="mid", name=f"av{i}_{2 * jp + u}"
                                )
                        return emit

                    def make_c(i, jp, avs, c):
                        def emit():
                            # Diagonal chunk v: only columns >= 128*v are live;
                            # the [128,128] block at [128v, 128(v+1)) is
                            # triangular, everything right of it fully live.
                            diag = c >= 4 * i
                            v = c - 4 * i if diag else 0
                            off = P * v if diag else 0
                            sc = ps.tile([P, 2 * IB], FP, tag="big", name=f"sc{i}_{jp}_{c}")
                            pe_mask = diag
                            for u in range(2):
                                ro = DH * u
                                nc.tensor.matmul(
                                    sc[:, IB * u + off:IB * (u + 1)],
                                    kt_sb[jp][ro:ro + DH, LCH * c:LCH * (c + 1)],
                                    qt_sb[jp][ro:ro + DH, IB * i + off:IB * (i + 1)],
                                    start=True, stop=not pe_mask,
                                    skip_group_check=True,
                                )
                                if pe_mask:
                                    # scores[tri] += -3e4 where q' < l: masked
                                    # exp underflows to 0, keeping the gpsimd
                                    # affine_select out of the last block's
                                    # latency-critical chains
                                    nc.tensor.matmul(
                                        sc[:, IB * u + off:IB * u + off + P],
                                        negi, utri,
                                        start=False, stop=True,
                                        skip_group_check=True,
                                    )
                            ex = exp_pool.tile([P, 2 * IB], BF, tag="ex", name=f"ex{i}_{jp}_{c}")
                            sc_g = sc.rearrange("p (u n) -> p u n", u=2)
                            ex_g = ex.rearrange("p (u n) -> p u n", u=2)
                            nc.scalar.activation(
                                out=ex_g[:, :, off:], in_=sc_g[:, :, off:],
                                func=mybir.ActivationFunctionType.Exp,
                            )
                            if diag and not pe_mask:
                                # blocks 0-2: zero the [128,128] triangle on
                                # the idle Pool engine (latency hides under
                                # the interleaved fillers)
                                for u in range(2):
                                    nc.gpsimd.affine_select(
                                        out=ex[:, IB * u + off:IB * u + off + P],
                                        in_=ex[:, IB * u + off:IB * u + off + P],
                                        compare_op=mybir.AluOpType.is_ge,
                                        fill=0.0, base=0,
                                        channel_multiplier=-1, pattern=[[1, P]],
                                    )
                            for u in range(2):
                                h = 2 * jp + u
                                nc.tensor.matmul(
                                    avs[u][0:VA, off:],
                                    vaug[:, VA * (HL * c + h): VA * (HL * c + h + 1)],
                                    ex[:, IB * u + off:IB * (u + 1)],
                                    start=(c == 0), stop=(c == nch - 1),
                                    skip_group_check=True,
                                )
                        return emit

                    def make_tail(i, jp, avs, u, w):
                        """Normalize the 128-query column slice w of i-block i:
                        avs columns [128w, 128(w+1)) are final as soon as diag
                        chunk v=w has accumulated, so these chains hide under
                        the remaining diagonal chunks instead of trailing the
                        whole block."""
                        def emit():
                            h = 2 * jp + u
                            ro = DH * u
                            cw = P * w
                            recip = nrm_pool.tile([1, P], FP, tag="rc", name=f"rc{i}_{h}_{w}")
                            nc.vector.reciprocal(
                                out=recip, in_=avs[u][DH:DH + 1, cw:cw + P]
                            )
                            bc = nrm_pool.tile([DH, P], FP, tag="bc", name=f"bc{i}_{h}_{w}")
                            nc.gpsimd.partition_broadcast(out_ap=bc, in_ap=recip)
                            nc.vector.tensor_mul(
                                out=ao_sb[jp][ro:ro + DH, IB * i + cw:IB * i + cw + P],
                                in0=avs[u][0:DH, cw:cw + P], in1=bc,
                            )
                        return emit

                    units.append(make_pair_start(i, jp, avs))
                    for c in range(nch):
                        units.append(make_c(i, jp, avs, c))
                        if c >= 4 * i:
                            w = c - 4 * i
                            units.append(make_tail(i, jp, avs, 0, w))
                            units.append(make_tail(i, jp, avs, 1, w))
                            if after_slice is not None and jp == jp_order[-1]:
                                units += after_slice(w)
                return units

            def y_units(i, j_order=(0, 1), split_dma=False):
                units = []

                def make(tt):
                    def emit():
                        ysb = ysb_pool.tile([P, D], BF, tag="ysb", name=f"ysb{tt}")
                        for db in range(2):
                            yps = ps.tile([P, IB], FP, tag="fil", name=f"yps{tt}_{db}")
                            for k, j in enumerate(j_order):
                                nc.tensor.matmul(
                                    yps,
                                    ao_sb[j][:, P * tt:P * (tt + 1)],
                                    wo_sb[:, D * j + IB * db:D * j + IB * (db + 1)],
                                    start=(k == 0), stop=(k == 1),
                                )
                            # gpsimd cannot read PSUM on real HW: alternate the
                            # psum->sbuf copies between ACT and DVE; the final
                            # block's tail tiles go to ACT (idle after last exp)
                            if (split_dma and tt % 4 >= 2) or (tt + db) % 2 == 0:
                                nc.scalar.activation(
                                    out=ysb[:, IB * db:IB * (db + 1)], in_=yps,
                                    func=mybir.ActivationFunctionType.Copy,
                                )
                            else:
                                nc.vector.tensor_copy(
                                    out=ysb[:, IB * db:IB * (db + 1)], in_=yps,
                                )
                            if split_dma:
                                nc.sync.dma_start(
                                    out=y[P * tt:P * (tt + 1), IB * db:IB * (db + 1)],
                                    in_=ysb[:, IB * db:IB * (db + 1)],
                                )
                        if not split_dma:
                            nc.sync.dma_start(out=y[P * tt:P * (tt + 1), :], in_=ysb)
                    return emit
                for tt in range(4 * i, 4 * (i + 1)):
                    units.append(make(tt))
                return units

            def interleave(main, fillers, frac=1.0):
                """Emit `main` units with `fillers` spread evenly between
                them; `frac` < 1 finishes the fillers after that fraction of
                the main units (keeps filler DMAs off the critical tail)."""
                if not main:
                    for f in fillers:
                        f()
                    return
                nf = len(fillers)
                span = max(1, int(len(main) * frac))
                fi = 0
                for k, m in enumerate(main):
                    m()
                    want = min(nf, (k + 1) * nf // span)
                    while fi < want:
                        fillers[fi]()
                        fi += 1
                while fi < nf:
                    fillers[fi]()
                    fi += 1

            # ---------- pipelined schedule ----------
            # NOTE: Tile is a *tracing* scheduler — emission order defines the
            # dataflow. Every consumer must be emitted after its producer, so
            # block-0 setup runs as a strict prologue.
            for u in proj0_units():
                u()
            for u in v_units(0):
                u()

            for tb in range(NTB):
                fillers = []
                if tb + 1 < NTB:
                    fillers += proj_units(tb + 1)
                # y(0) into block 1; y(1) and y(2) into block 3 (block 2 is
                # already PE-bound with proj(3); block 3 has the most
                # ACT-bound slack to fill)
                if tb == 1:
                    fillers += y_units(0)
                elif tb == 3:
                    fillers += y_units(1) + y_units(2)
                # last block: run jp=1 first so ao_sb[1] is ready early and
                # the final y accumulation (j=1 first) can start before jp=0's
                # normalize tail completes
                jp_order = (1, 0) if tb == NTB - 1 else (0, 1)
                interleave(attn_units(tb, jp_order), fillers)
            for u in y_units(NTB - 1, j_order=(1, 0), split_dma=True):
                u()

    nc.compile()
    return nc


def get_nc():
    if "nc" not in _CACHE:
        _CACHE["nc"] = build_nc()
    return _CACHE["nc"]


def kernel(x, wq, bq, wk, bk, wv, bv, wo, bo):
    import ml_dtypes
    bf16 = ml_dtypes.bfloat16

    x = np.asarray(x, dtype=np.float32)
    wq = np.asarray(wq, dtype=np.float32)
    wk = np.asarray(wk, dtype=np.float32)
    wv = np.asarray(wv, dtype=np.float32)
    wo = np.asarray(wo, dtype=np.float32)
    bq = np.asarray(bq, dtype=np.float32)
    bk = np.asarray(bk, dtype=np.float32)
    bv = np.asarray(bv, dtype=np.float32)
    bo = np.asarray(bo, dtype=np.float32)

    nc = get_nc()
    in_maps = []
    for core in range(NCORES):
        b, g = divmod(core, GROUPS)
        cs = slice(JC * g, JC * (g + 1))
        in_maps.append({
            "x": np.ascontiguousarray(x[b].astype(bf16).T),
            "wq": np.ascontiguousarray(wq[:, cs].astype(bf16)),
            "wk": np.ascontiguousarray(wk[:, cs].astype(bf16)),
            "wv": np.ascontiguousarray(wv[:, cs].astype(bf16)),
            "wo": np.ascontiguousarray(wo[cs, :].astype(bf16)),
            "bq": np.ascontiguousarray(bq[cs].reshape(2, P).T),
            "bk": np.ascontiguousarray(bk[cs].reshape(2, P).T),
        })
    res = run_bass_kernel_spmd(nc, in_maps, list(range(NCORES)))
    _CACHE["last_results"] = res

    out = np.zeros((B, S, D), np.float32)
    for core in range(NCORES):
        out[core // GROUPS] += np.asarray(res.results[core]["y"], dtype=np.float32)
    # bv and bo never pass through softmax nonlinearity: rows of attn sum to 1,
    # so (v + bv) contributes exactly bv @ wo to every output row.
    out += (bv @ wo + bo)[None, None, :]
    return out
